# revision 1
# baseline (speedup 1.0000x reference)
"""GatedDeltaNet attention kernel for 8 Trainium2 NeuronCores.

Problem: B=2, L=2048, D=1024, H=16 heads (Dh=64).
  q,k,v = x@Wq, x@Wk, x@Wv ; beta = sigmoid(x@Wb + bb)
  q,k l2-normalized per head; out[l] = sum_{t<=l} beta_t <qh_l,kh_t> vh_t
  y = out @ Wo

Sharding: 8 cores = 2 batches x 4 head-groups (4 heads each). Each core
computes its batch/heads slice end-to-end including a partial y (contraction
over its 256 Wo rows); host sums the 4 partials per batch.

Device algorithm (per core), all matmuls in float32r (full PE rate):
  P1: qT/kT = W^T-style projections into [d', l] layout (lhsT=W, rhs=xT),
      v into [t, e] layout (lhsT=x-block, rhs=Wvb with beta logits fused as
      4 extra columns). l2-norm factors via Square + selector-matmul
      partition reductions; 1/|k_t| and beta fold into v ("vtilde"),
      1/|q_l| folds into the attention-output copy, with its per-head
      row-broadcast materialized by K=1 indicator matmuls on TensorE.
  P2: per head-pair, causal-skipped score tiles ST[t,l] (2-head packed via
      row tile_position), PSUM->SBUF copy is plain (off-diag) or one
      mask-multiply (diagonal); out2 accumulated over t-blocks with 2-head
      column packing.
  P3: yT = Wo^T @ attnT, copied to SBUF and DMA'd out.
"""

import numpy as np

P = 128
L = 2048
D = 1024
H = 16
KS = D // P        # 8 contraction subtiles
NT = L // P        # 16 t-blocks
CH = 512
NCH = L // CH      # 4 l-chunks
DH = 64
HC = 4             # heads per core
NP = HC // 2       # head pairs per core
NCORES = 8
GROUPS = NCORES // 2  # head groups (4)

_CACHE = {}

# out2 (score @ v) matmuls in bf16: enables 2-head column packing on the
# PE array (fp32r matmuls cannot write PSUM at partition 64).
OUT2_BF16 = True
DEBUG_DUMP = False


def _build_nc():
    import concourse.bass as bass  # noqa: F401
    import concourse.tile as tile
    import concourse.mybir as mybir
    from concourse import bacc
    from contextlib import ExitStack

    F32 = mybir.dt.float32
    F32R = mybir.dt.float32r
    AF = mybir.ActivationFunctionType
    OP = mybir.AluOpType

    nc = bacc.Bacc(
        "TRN2", target_bir_lowering=False, debug=False, num_devices=NCORES
    )

    xT = nc.dram_tensor("xT", [KS, P, L], F32R, kind="ExternalInput")
    wq = nc.dram_tensor("wq", [KS, P, NP * P], F32R, kind="ExternalInput")
    wk = nc.dram_tensor("wk", [KS, P, NP * P], F32R, kind="ExternalInput")
    wvb = nc.dram_tensor("wvb", [KS, P, HC * DH + HC], F32R, kind="ExternalInput")
    wo = nc.dram_tensor("wo", [NP, P, D], F32R, kind="ExternalInput")
    sel = nc.dram_tensor("sel", [P, 2], F32R, kind="ExternalInput")
    bbb = nc.dram_tensor("bbb", [P, HC], F32, kind="ExternalInput")
    masks = nc.dram_tensor("masks", [P, P], F32, kind="ExternalInput")
    ind = nc.dram_tensor("ind", [2, P], F32R, kind="ExternalInput")
    yT = nc.dram_tensor("yT", [D, L], F32, kind="ExternalOutput")
    dbg = {}
    if DEBUG_DUMP:
        BF16_ = mybir.dt.bfloat16
        DT2_ = BF16_ if OUT2_BF16 else F32R
        for nm, shp, dt_ in [("d_qT0", [P, L], F32R), ("d_kT0", [P, L], F32R),
                        ("d_vt", [P, NT * HC * DH], DT2_), ("d_factor", [P, NT * HC], F32),
                        ("d_rnq0", [1, L], F32), ("d_rnqb0", [P, L], F32),
                        ("d_attnT0", [P, L], F32R), ("d_rnk", [P, NT * 2 * NP], F32)]:
            dbg[nm] = nc.dram_tensor(nm, shp, dt_, kind="ExternalOutput")

    NV = HC * DH  # 256

    with tile.TileContext(nc) as tc:
        with ExitStack() as ctx:
            pconst = ctx.enter_context(tc.tile_pool(name="const", bufs=1))
            pmain = ctx.enter_context(tc.tile_pool(name="main", bufs=1))

            # running modeled-busy accumulators for DVE vs ACT copy balancing
            eng_load = {"dve": 0.0, "act": 0.0}

            def note(eng, t):
                eng_load[eng] += t

            def bal_copy(out_ap, in_ap, fd):
                cd = (120 + fd) / 0.96
                ca = (210 + fd) / 1.05
                if eng_load["dve"] + cd <= eng_load["act"] + ca:
                    note("dve", cd)
                    nc.vector.tensor_copy(out_ap, in_ap)
                else:
                    note("act", ca)
                    nc.scalar.activation(out_ap, in_ap, AF.Copy)

            sel_sb = pconst.tile([P, 2], F32R, tag="sel", name="sel")
            nc.sync.dma_start(sel_sb[:], sel.ap())
            bbb_sb = pconst.tile([P, HC], F32, tag="bbb", name="bbb")
            nc.sync.dma_start(bbb_sb[:], bbb.ap())
            mask_sb = pconst.tile([P, P], F32, tag="mask", name="mask")
            wo_sb = pconst.tile([P, NP, D], F32R, tag="wo", name="wo")
            ind_sb = pconst.tile([2, P], F32R, tag="ind", name="ind")
            # (their DMAs are issued after the projection inputs below —
            # they are not needed until P2/P3)

            qT = [pmain.tile([P, L], F32R, tag=f"qT{p}", name=f"qT{p}") for p in range(NP)]
            kT = [pmain.tile([P, L], F32R, tag=f"kT{p}", name=f"kT{p}") for p in range(NP)]
            BF16 = mybir.dt.bfloat16
            DT2 = BF16 if OUT2_BF16 else F32R
            vt = pmain.tile([P, NT, NV], DT2, tag="vt", name="vt")
            factor = pmain.tile([P, NT, HC], F32, tag="factor", name="factor")
            rnk_sb = pmain.tile([P, NT, 2 * NP], F32, tag="rnk", name="rnk")
            # one [2, L] tile per head-pair (head rows at partitions 0/1)
            rnq_sb = [pmain.tile([2, L], F32R, tag=f"rnq{p}", name=f"rnq{p}") for p in range(NP)]

            # ---------------- P1: projections ----------------
            with ExitStack() as p1:
                px = p1.enter_context(tc.tile_pool(name="x", bufs=KS))
                pw = p1.enter_context(tc.tile_pool(name="w", bufs=1))
                psq = p1.enter_context(tc.tile_pool(name="sq", bufs=3))
                ptmp = p1.enter_context(tc.tile_pool(name="tmp", bufs=4))
                ppA = p1.enter_context(
                    tc.tile_pool(name="ppA", bufs=5, space="PSUM")
                )
                ppB = p1.enter_context(
                    tc.tile_pool(name="ppB", bufs=2, space="PSUM")
                )
                ppC = p1.enter_context(
                    tc.tile_pool(name="ppC", bufs=1, space="PSUM")
                )
                # all 32 k-norm selector matmuls land in one PSUM bank as
                # [128, tb, pair*2+h] columns; overwrite-on-pending-zero
                # makes disjoint-column writes of one started group safe
                ssk_all = ppC.tile([P, NT, 2 * NP], F32, tag="sskall", name="sskall")
                ssk_n = [0]

                # Fine-grained input DMAs so matmul deps release early
                # (byte-range dep tracking): per-ksub weight slices and
                # per-chunk x slices, interleaved in consumption order and
                # spread across the SP/ACT/GPSIMD DMA queues.
                wq_sb = pw.tile([P, KS, NP * P], F32R, tag="wq", name="wq")
                wk_sb = pw.tile([P, KS, NP * P], F32R, tag="wk", name="wk")
                wvb_sb = pw.tile([P, KS, NV + HC], F32R, tag="wvb", name="wvb")
                x_sb = [px.tile([P, L], F32R, tag="x", name="x")
                        for _ in range(KS)]
                x_eng = [nc.sync, nc.scalar]
                for ks in range(KS):
                    nc.gpsimd.dma_start(wq_sb[:, ks, :], wq.ap()[ks])
                    for c in range(NCH):
                        x_eng[(ks * NCH + c) % 2].dma_start(
                            x_sb[ks][:, c * CH:(c + 1) * CH],
                            xT.ap()[ks][:, c * CH:(c + 1) * CH],
                        )
                for ks in range(KS):
                    nc.gpsimd.dma_start(wk_sb[:, ks, :], wk.ap()[ks])
                    nc.gpsimd.dma_start(wvb_sb[:, ks, :], wvb.ap()[ks])

                # deferred P2/P3 constants
                nc.gpsimd.dma_start(ind_sb[:], ind.ap())
                nc.gpsimd.dma_start(mask_sb[:], masks.ap())
                nc.gpsimd.dma_start(wo_sb[:], wo.ap().rearrange("s p d -> p s d"))

                # q and k projections, with norm-factor chains
                for w_sb, dst, is_q in ((wq_sb, qT, True), (wk_sb, kT, False)):
                    for pair in range(NP):
                        ps = [ppA.tile([P, CH], F32, tag="mm", name="mm") for _ in range(NCH)]
                        for ks in range(KS):
                            lhsT = w_sb[:, ks, pair * P:(pair + 1) * P]
                            for c in range(NCH):
                                nc.tensor.matmul(
                                    ps[c][:],
                                    lhsT,
                                    x_sb[ks][:, c * CH:(c + 1) * CH],
                                    start=(ks == 0),
                                    stop=(ks == KS - 1),
                                )
                        for c in range(NCH):
                            bal_copy(
                                dst[pair][:, c * CH:(c + 1) * CH], ps[c][:], CH
                            )
                            sq = psq.tile([P, CH], F32R, tag="sq", name="sq")
                            nc.scalar.activation(sq[:], ps[c][:], AF.Square)
                            note("act", (172 + CH) / 1.2)
                            if is_q:
                                ss = ppB.tile([2, CH], F32, tag="ss", name="ss")
                                nc.tensor.matmul(
                                    ss[:], sel_sb[:], sq[:],
                                    start=True, stop=True,
                                )
                                nc.scalar.activation(
                                    rnq_sb[pair][:, c * CH:(c + 1) * CH],
                                    ss[:], AF.Abs_reciprocal_sqrt,
                                )
                                note("act", (172 + CH) / 1.2)
                            else:
                                for tr in range(CH // P):
                                    tb = c * (CH // P) + tr
                                    nc.tensor.matmul(
                                        ssk_all[:, tb, pair * 2:pair * 2 + 2],
                                        sq[:, tr * P:(tr + 1) * P],
                                        sel_sb[:],
                                        start=(ssk_n[0] == 0),
                                        stop=(ssk_n[0] == 2 * NP * NT - 1),
                                        skip_group_check=True,
                                    )
                                    ssk_n[0] += 1

                # k-norm: single reciprocal + sqrt over the packed bank
                nc.scalar.activation(
                    rnk_sb[:].rearrange("p a b -> p (a b)"),
                    ssk_all[:].rearrange("p a b -> p (a b)"),
                    AF.Abs_reciprocal_sqrt)
                note("act", 300.0)

                # v projection (+ fused beta logits) -> vtilde, plus a
                # second GEMM off the same stationary x-block producing k in
                # normal [t, d] layout (bf16) for the inter-chunk state path
                kn = pmain.tile([P, NT, NV], BF16, tag="kn", name="kn")
                for tb in range(NT):
                    psv = ppA.tile([P, NV + HC], F32, tag="mm", name="mm")
                    need_kn = tb < NT - NT // NCH  # chunk 3 never enters the state
                    if need_kn:
                        psk = ppA.tile([P, NV], F32, tag="mm", name="mmk")
                    for ks in range(KS):
                        nc.tensor.matmul(
                            psv[:],
                            x_sb[ks][:, tb * P:(tb + 1) * P],
                            wvb_sb[:, ks, :],
                            start=(ks == 0),
                            stop=(ks == KS - 1),
                        )
                        if need_kn:
                            nc.tensor.matmul(
                                psk[:],
                                x_sb[ks][:, tb * P:(tb + 1) * P],
                                wk_sb[:, ks, :],
                                start=(ks == 0),
                                stop=(ks == KS - 1),
                            )
                    if need_kn:
                        bal_copy(kn[:, tb, :], psk[:], NV)
                    bl = ptmp.tile([P, HC], F32, tag="bl", name="bl")
                    nc.vector.tensor_tensor(
                        bl[:], psv[:, NV:], bbb_sb[:], OP.add
                    )
                    bs = ptmp.tile([P, HC], F32, tag="bs", name="bs")
                    nc.scalar.activation(bs[:], bl[:], AF.Sigmoid)
                    note("act", 180.0)
                    note("dve", 300.0)
                    nc.vector.tensor_tensor(
                        factor[:, tb, :], bs[:], rnk_sb[:, tb, :], OP.mult
                    )
                    nc.vector.tensor_tensor(
                        vt[:, tb, :].rearrange("p (h e) -> p h e", e=DH),
                        psv[:, :NV].rearrange("p (h e) -> p h e", e=DH),
                        factor[:, tb, :, None].to_broadcast((P, HC, DH)),
                        OP.mult,
                    )
                    note("dve", (120 + NV) / 0.96)

            # ---------------- P2 + P3 ----------------
            with ExitStack() as p2:
                p2m = p2.enter_context(tc.tile_pool(name="p2m", bufs=1))
                pst = p2.enter_context(tc.tile_pool(name="stbuf", bufs=8))
                pyout = p2.enter_context(tc.tile_pool(name="yout", bufs=6))
                ppst = p2.enter_context(
                    tc.tile_pool(name="ppst", bufs=4, space="PSUM")
                )
                ppo2 = p2.enter_context(
                    tc.tile_pool(name="ppo2", bufs=2, space="PSUM")
                )
                pps_s = p2.enter_context(
                    tc.tile_pool(name="pps_s", bufs=1, space="PSUM")
                )

                rnqb = [p2m.tile([P, L], F32, tag=f"rnqb{p}", name=f"rnqb{p}") for p in range(NP)]
                attnT = [p2m.tile([P, L], F32R, tag=f"attnT{p}", name=f"attnT{p}") for p in range(NP)]
                # broadcast rnq rows across partitions via two K=1
                # accumulating matmuls against host indicator rows
                for pair in range(NP):
                    for c in range(NCH):
                        bc = ppst.tile([P, CH], F32, tag="st", name="bc")
                        nc.tensor.matmul(
                            bc[:],
                            ind_sb[:],
                            rnq_sb[pair][:, c * CH:(c + 1) * CH],
                            start=True, stop=True,
                        )
                        bal_copy(rnqb[pair][:, c * CH:(c + 1) * CH], bc[:], CH)

                # running DeltaNet state S[d, e] per pair (accumulated in
                # PSUM across chunk boundaries) + bf16 copies of S and qT
                # for the inter-chunk matmuls
                s_ps = [pps_s.tile([P, NV], F32, tag=f"sps{p}", name=f"sps{p}")
                        for p in range(NP)]
                s_sb = [p2m.tile([P, NV], BF16, tag=f"ssb{p}", name=f"ssb{p}")
                        for p in range(NP)]
                qTb = [p2m.tile([P, L], BF16, tag=f"qTb{p}", name=f"qTb{p}")
                       for p in range(NP)]
                for pair in range(NP):
                    for c in range(1, NCH):
                        bal_copy(qTb[pair][:, c * CH:(c + 1) * CH],
                                 qT[pair][:, c * CH:(c + 1) * CH], CH)

                for c in range(NCH):
                    if c > 0:
                        # fold chunk c-1 into the state, snapshot to bf16
                        for pair in range(NP):
                            for tsub in range(4):
                                tb = (c - 1) * 4 + tsub
                                nc.tensor.matmul(
                                    s_ps[pair][:],
                                    kn[:, tb, pair * P:(pair + 1) * P],
                                    vt[:, tb, :],
                                    start=(c == 1 and tsub == 0),
                                    stop=(c == NCH - 1 and tsub == 3),
                                    skip_group_check=True,
                                )
                            bal_copy(s_sb[pair][:], s_ps[pair][:], NV)
                    for pair in range(NP):
                        o2 = ppo2.tile([P, CH], F32, tag="o2", name="o2")
                        if c > 0:
                            # inter-chunk contribution: o2 = S_h^T-applied q
                            for hh in range(2):
                                h = 2 * pair + hh
                                nc.tensor.matmul(
                                    o2[64 * hh:64 * (hh + 1), :],
                                    s_sb[pair][
                                        64 * hh:64 * (hh + 1),
                                        h * DH:(h + 1) * DH,
                                    ],
                                    qTb[pair][
                                        64 * hh:64 * (hh + 1),
                                        c * CH:(c + 1) * CH,
                                    ],
                                    start=True, stop=False,
                                    tile_position=(64 * hh, 64 * hh),
                                    skip_group_check=True,
                                )
                        for T in range(4 * c, 4 * c + 4):
                            j = T - 4 * c
                            lo = P * j if j > 0 else 0
                            stps = [
                                ppst.tile([P, CH], F32, tag="st", name="st")
                                for _ in range(2)
                            ]
                            for hh in range(2):
                                nc.tensor.matmul(
                                    stps[hh][:, lo:CH],
                                    kT[pair][
                                        64 * hh:64 * (hh + 1), T * P:(T + 1) * P
                                    ],
                                    qT[pair][
                                        64 * hh:64 * (hh + 1),
                                        c * CH + lo:(c + 1) * CH,
                                    ],
                                    start=True, stop=True,
                                )
                            st_sb = [
                                pst.tile([P, CH], DT2, tag="st_sb", name="st_sb")
                                for _ in range(2)
                            ]
                            for hh in range(2):
                                    # triangular 128-col block at the causal
                                    # frontier; rest is plain copy
                                    nc.vector.tensor_tensor(
                                        st_sb[hh][:, lo:lo + P],
                                        stps[hh][:, lo:lo + P],
                                        mask_sb[:], OP.mult,
                                    )
                                    note("dve", (120 + P) / 0.96)
                                    if lo + P < CH:
                                        bal_copy(
                                            st_sb[hh][:, lo + P:CH],
                                            stps[hh][:, lo + P:CH],
                                            CH - lo - P,
                                        )
                            for hh in range(2):
                                h = 2 * pair + hh
                                nc.tensor.matmul(
                                    o2[64 * hh:64 * (hh + 1), lo:CH],
                                    vt[:, T, h * DH:(h + 1) * DH],
                                    st_sb[hh][:, lo:CH],
                                    start=(c == 0 and T == 0),
                                    stop=(T == 4 * c + 3),
                                    tile_position=(0, 64 * hh),
                                    skip_group_check=True,
                                )
                        nc.vector.tensor_tensor(
                            attnT[pair][:, c * CH:(c + 1) * CH],
                            o2[:],
                            rnqb[pair][:, c * CH:(c + 1) * CH],
                            OP.mult,
                        )
                        note("dve", (120 + CH) / 0.96)

                    # P3 for this chunk: yT[:, c] = wo^T @ attnT[:, c]
                    for m in range(D // P):
                        py = ppst.tile([P, CH], F32, tag="st", name="y")
                        for pair in range(NP):
                            nc.tensor.matmul(
                                py[:],
                                wo_sb[:, pair, m * P:(m + 1) * P],
                                attnT[pair][:, c * CH:(c + 1) * CH],
                                start=(pair == 0),
                                stop=(pair == NP - 1),
                            )
                        yo = pyout.tile([P, CH], F32, tag="yo", name="yo")
                        bal_copy(yo[:], py[:], CH)
                        nc.sync.dma_start(
                            yT.ap()[m * P:(m + 1) * P, c * CH:(c + 1) * CH],
                            yo[:],
                        )

                if DEBUG_DUMP:
                    nc.sync.dma_start(dbg["d_qT0"].ap(), qT[0][:])
                    nc.sync.dma_start(dbg["d_kT0"].ap(), kT[0][:])
                    nc.sync.dma_start(dbg["d_vt"].ap(), vt[:].rearrange("p a b -> p (a b)"))
                    nc.sync.dma_start(dbg["d_factor"].ap(), factor[:].rearrange("p a b -> p (a b)"))
                    nc.sync.dma_start(dbg["d_rnq0"].ap(), rnq_sb[0][:])
                    nc.sync.dma_start(dbg["d_rnqb0"].ap(), rnqb[0][:])
                    nc.sync.dma_start(dbg["d_attnT0"].ap(), attnT[0][:])
                    nc.sync.dma_start(dbg["d_rnk"].ap(), rnk_sb[:].rearrange("p a b -> p (a b)"))

    nc.compile()
    return nc


def get_nc():
    if "nc" not in _CACHE:
        _CACHE["nc"] = _build_nc()
    return _CACHE["nc"]


def make_core_inputs(x, Wq, Wk, Wv, Wo, Wb, bb):
    """Build the 8 per-core input maps from full inputs."""
    x = np.asarray(x, dtype=np.float32)
    Wq = np.asarray(Wq, dtype=np.float32)
    Wk = np.asarray(Wk, dtype=np.float32)
    Wv = np.asarray(Wv, dtype=np.float32)
    Wo = np.asarray(Wo, dtype=np.float32)
    Wb = np.asarray(Wb, dtype=np.float32)
    bb = np.asarray(bb, dtype=np.float32)

    selm = np.zeros((P, 2), dtype=np.float32)
    selm[:64, 0] = 1.0
    selm[64:, 1] = 1.0
    indm = np.zeros((2, P), dtype=np.float32)
    indm[0, :64] = 1.0
    indm[1, 64:] = 1.0
    masks = (np.arange(P)[:, None] <= np.arange(P)[None, :]).astype(np.float32)

    in_maps = []
    for core in range(NCORES):
        b, g = divmod(core, GROUPS)
        hs = slice(NV_G * g, NV_G * (g + 1))
        bs = slice(HC * g, HC * (g + 1))
        xTc = np.ascontiguousarray(x[b].T).reshape(KS, P, L)
        wqc = np.ascontiguousarray(Wq[:, hs]).reshape(KS, P, NP * P)
        wkc = np.ascontiguousarray(Wk[:, hs]).reshape(KS, P, NP * P)
        wvbc = np.ascontiguousarray(
            np.concatenate([Wv[:, hs], Wb[:, bs]], axis=1)
        ).reshape(KS, P, NV_G + HC)
        woc = np.ascontiguousarray(Wo[hs, :]).reshape(NP, P, D)
        bbbc = np.ascontiguousarray(np.tile(bb[bs][None, :], (P, 1)))
        in_maps.append(
            {
                "xT": xTc,
                "wq": wqc,
                "wk": wkc,
                "wvb": wvbc,
                "wo": woc,
                "sel": selm,
                "bbb": bbbc,
                "masks": masks,
                "ind": indm,
            }
        )
    return in_maps


NV_G = HC * DH  # 256 columns per head group


def kernel(x, Wq, Wk, Wv, Wo, Wb, bb):
    from concourse.bass_utils import run_bass_kernel_spmd

    nc = get_nc()
    in_maps = make_core_inputs(x, Wq, Wk, Wv, Wo, Wb, bb)
    try:
        res = run_bass_kernel_spmd(nc, in_maps, core_ids=list(range(NCORES)))
    except Exception:
        # transient NRT wedges (e.g. NRT_EXEC_UNIT_UNRECOVERABLE) clear on
        # a fresh attempt; retry once before giving up
        res = run_bass_kernel_spmd(nc, in_maps, core_ids=list(range(NCORES)))
    B = 2
    y = np.zeros((B, L, D), dtype=np.float32)
    for core in range(NCORES):
        b = core // GROUPS
        y[b] += res.results[core]["yT"].T
    return y


if __name__ == "__main__":
    rng = np.random.default_rng(0)
    ins = {
        "x": rng.standard_normal((2, L, D)).astype(np.float32),
        "Wq": (0.02 * rng.standard_normal((D, D))).astype(np.float32),
        "Wk": (0.02 * rng.standard_normal((D, D))).astype(np.float32),
        "Wv": (0.02 * rng.standard_normal((D, D))).astype(np.float32),
        "Wo": (0.02 * rng.standard_normal((D, D))).astype(np.float32),
        "Wb": (0.02 * rng.standard_normal((D, H))).astype(np.float32),
        "bb": np.zeros(H, dtype=np.float32),
    }
    out = kernel(**ins)
    print("kernel ran, out shape", out.shape, "mean abs", np.abs(out).mean())



# revision 7
# speedup vs baseline: 1.2323x; 1.2323x over previous
"""GatedDeltaNet attention kernel for 8 Trainium2 NeuronCores.

Problem: B=2, L=2048, D=1024, H=16 heads (Dh=64).
  q,k,v = x@Wq, x@Wk, x@Wv ; beta = sigmoid(x@Wb + bb)
  q,k l2-normalized per head; out[l] = sum_{t<=l} beta_t <qh_l,kh_t> vh_t
  y = out @ Wo

Sharding: 8 cores = 2 batches x 4 head-groups (4 heads each). Each core
computes its batch/heads slice end-to-end including a partial y (contraction
over its 256 Wo rows); host sums the 4 bf16 partials per batch.

All GEMM operands are bf16 (PSUM accumulation in f32), which halves DMA
volume and keeps every matmul at 1 cycle/row regardless of tile width.

Device algorithm (per core):
  P1: qT/kT = W^T-style projections into [d, l] layout (lhsT=W slice,
      rhs=xT), accumulated over 8 K-subtiles in 512-col PSUM chunks; the
      first q chunks stream behind the x DMAs. v is projected into [t, e]
      layout with the beta logits fused as 4 extra columns. l2-norm factors
      via ACT Square + selector matmuls; 1/|k_t| and beta fold into v
      ("vtilde"); 1/|q_l| is broadcast across partitions with K=2 indicator
      matmuls and multiplied into the attention output. k's [t, d] layout
      (kn, needed for the state updates) comes from DMA block-transposes of
      kT -- no second GEMM.
  P2: 128-wide chunks. Per chunk: per-head diagonal score tile (64-wide
      contraction), triangular mask folded into the mandatory PSUM->SBUF
      copy, out2 accumulation = one full-width inter-chunk matmul against
      the block-diagonal state S plus two 64-partition intra matmuls
      (tile_position column packing). State S[d,e] per pair is accumulated
      in PSUM as two diagonal 64x64 blocks (cross-head blocks stay zero via
      a one-time memset), snapshotted to SBUF bf16 once per chunk.
  P3: yT = Wo^T @ attnT per 512-superchunk, copied to bf16 and DMA'd out.
"""

import numpy as np
import ml_dtypes

P = 128
L = 2048
D = 1024
H = 16
KS = D // P        # 8 contraction subtiles
NT = L // P        # 16 t-blocks / P2 chunks
CH = 512
NCH = L // CH      # 4 l-superchunks
DH = 64
HC = 4             # heads per core
NP = HC // 2       # head pairs per core
NV = HC * DH       # 256 v columns per core
NCORES = 8
GROUPS = NCORES // 2

_CACHE = {}


def _build_nc():
    import concourse.bass as bass  # noqa: F401
    import concourse.tile as tile
    import concourse.mybir as mybir
    from concourse import bacc
    from contextlib import ExitStack

    F32 = mybir.dt.float32
    BF16 = mybir.dt.bfloat16
    AF = mybir.ActivationFunctionType
    OP = mybir.AluOpType

    nc = bacc.Bacc(
        "TRN2", target_bir_lowering=False, debug=False, num_devices=NCORES
    )

    xT = nc.dram_tensor("xT", [KS, P, L], BF16, kind="ExternalInput")
    wq = nc.dram_tensor("wq", [KS, P, NP * P], BF16, kind="ExternalInput")
    wk = nc.dram_tensor("wk", [KS, P, NP * P], BF16, kind="ExternalInput")
    wvb = nc.dram_tensor("wvb", [KS, P, NV + HC], BF16, kind="ExternalInput")
    wo = nc.dram_tensor("wo", [NP, P, D], BF16, kind="ExternalInput")
    sel = nc.dram_tensor("sel", [P, 2], BF16, kind="ExternalInput")
    bbb = nc.dram_tensor("bbb", [P, HC], F32, kind="ExternalInput")
    masks = nc.dram_tensor("masks", [P, P], F32, kind="ExternalInput")
    ind = nc.dram_tensor("ind", [2, P], BF16, kind="ExternalInput")
    yT = nc.dram_tensor("yT", [D, L], BF16, kind="ExternalOutput")

    with tile.TileContext(nc) as tc:
        with ExitStack() as ctx:
            pconst = ctx.enter_context(tc.tile_pool(name="const", bufs=1))
            pmain = ctx.enter_context(tc.tile_pool(name="main", bufs=1))

            # alternate DVE/ACT for PSUM->SBUF traffic (GPSIMD cannot
            # read PSUM)
            def copy_any(i, out_ap, in_ap):
                if i % 2 == 0:
                    nc.vector.tensor_copy(out_ap, in_ap)
                else:
                    nc.scalar.activation(out_ap, in_ap, AF.Copy)

            sel_sb = pconst.tile([P, 2], BF16, tag="sel", name="sel")
            bbb_sb = pconst.tile([P, HC], F32, tag="bbb", name="bbb")
            mask_sb = pconst.tile([P, P], F32, tag="mask", name="mask")
            ind_sb = pconst.tile([2, P], BF16, tag="ind", name="ind")
            wo_sb = pconst.tile([P, NP, D], BF16, tag="wo", name="wo")

            # small consts via SWDGE on the (idle) Pool queue
            nc.gpsimd.dma_start(sel_sb[:], sel.ap())
            nc.gpsimd.dma_start(bbb_sb[:], bbb.ap())
            nc.gpsimd.dma_start(mask_sb[:], masks.ap())
            nc.gpsimd.dma_start(ind_sb[:], ind.ap())

            x_sb = [pmain.tile([P, L], BF16, tag=f"x{k}", name=f"x{k}")
                    for k in range(KS)]
            wq_sb = pmain.tile([P, KS, NP * P], BF16, tag="wq", name="wq")
            wk_sb = pmain.tile([P, KS, NP * P], BF16, tag="wk", name="wk")
            wvb_sb = pmain.tile([P, KS, NV + HC], BF16, tag="wvb", name="wvb")

            # input stream on the SP HWDGE queue, in consumption order:
            # wq first so the q GEMMs can chase the x slices as they land.
            nc.sync.dma_start(wq_sb[:], wq.ap().rearrange("s p d -> p s d"))
            nc.sync.dma_start(x_sb[0][:], xT.ap()[0])
            nc.sync.dma_start(wk_sb[:], wk.ap().rearrange("s p d -> p s d"))
            nc.sync.dma_start(x_sb[1][:], xT.ap()[1])
            nc.sync.dma_start(wvb_sb[:], wvb.ap().rearrange("s p d -> p s d"))
            for k in range(2, KS):
                nc.sync.dma_start(x_sb[k][:], xT.ap()[k])
            nc.sync.dma_start(wo_sb[:], wo.ap().rearrange("s p d -> p s d"))

            qT = [pmain.tile([P, L], BF16, tag=f"qT{p}", name=f"qT{p}")
                  for p in range(NP)]
            kT = [pmain.tile([P, L], BF16, tag=f"kT{p}", name=f"kT{p}")
                  for p in range(NP)]
            kn = [pmain.tile([P, NT, P], BF16, tag=f"kn{p}", name=f"kn{p}")
                  for p in range(NP)]
            vt = pmain.tile([P, NT, NV], BF16, tag="vt", name="vt")
            rnq_sb = [pmain.tile([2, L], BF16, tag=f"rnq{p}", name=f"rnq{p}")
                      for p in range(NP)]
            rnqb = [pmain.tile([P, L], BF16, tag=f"rnqb{p}", name=f"rnqb{p}")
                    for p in range(NP)]
            rnk_sb = pmain.tile([P, NT, 2 * NP], F32, tag="rnk", name="rnk")
            attnT = [pmain.tile([P, L], BF16, tag=f"attnT{p}", name=f"attnT{p}")
                     for p in range(NP)]

            # ---------------- P1: projections ----------------
            with ExitStack() as p1:
                psq = p1.enter_context(tc.tile_pool(name="sq", bufs=3))
                ptmp = p1.enter_context(tc.tile_pool(name="tmp", bufs=3))
                ppQK = p1.enter_context(
                    tc.tile_pool(name="ppQK", bufs=4, space="PSUM")
                )
                ppV = p1.enter_context(
                    tc.tile_pool(name="ppV", bufs=2, space="PSUM")
                )
                ppSS = p1.enter_context(
                    tc.tile_pool(name="ppSS", bufs=1, space="PSUM")
                )
                ppSSK = p1.enter_context(
                    tc.tile_pool(name="ppSSK", bufs=1, space="PSUM")
                )

                # all 32 k-norm selector matmuls land in one PSUM bank
                ssk_all = ppSSK.tile([P, NT, 2 * NP], F32, tag="ssk", name="ssk")
                ssk_n = [0]

                def finish_qk(ps, dst, pair, c, is_q, cp_i):
                    """Drain one [P, CH] projection chunk: bf16 copy + norms."""
                    copy_any(cp_i, dst[pair][:, c * CH:(c + 1) * CH], ps[:])
                    sq = psq.tile([P, CH], BF16, tag="sq", name="sq")
                    nc.scalar.activation(sq[:], ps[:], AF.Square)
                    if is_q:
                        ss = ppSS.tile([2, CH], F32, tag="ss", name="ss")
                        nc.tensor.matmul(
                            ss[:], sel_sb[:], sq[:], start=True, stop=True
                        )
                        nc.scalar.activation(
                            rnq_sb[pair][:, c * CH:(c + 1) * CH],
                            ss[:], AF.Abs_reciprocal_sqrt,
                        )
                    else:
                        for tr in range(CH // P):
                            tb = c * (CH // P) + tr
                            nc.tensor.matmul(
                                ssk_all[:, tb, pair * 2:pair * 2 + 2],
                                sq[:, tr * P:(tr + 1) * P],
                                sel_sb[:],
                                start=(ssk_n[0] == 0),
                                stop=(ssk_n[0] == 2 * NP * NT - 1),
                                skip_group_check=True,
                            )
                            ssk_n[0] += 1

                # q: chunks {0,1} stream behind the x DMAs, then {2,3}
                cp_i = 0
                for cg in range(2):
                    ps_q = {}
                    for c in (2 * cg, 2 * cg + 1):
                        for pair in range(NP):
                            ps_q[(c, pair)] = ppQK.tile(
                                [P, CH], F32, tag="qk", name=f"q{c}_{pair}"
                            )
                    for ks in range(KS):
                        for c in (2 * cg, 2 * cg + 1):
                            for pair in range(NP):
                                nc.tensor.matmul(
                                    ps_q[(c, pair)][:],
                                    wq_sb[:, ks, pair * P:(pair + 1) * P],
                                    x_sb[ks][:, c * CH:(c + 1) * CH],
                                    start=(ks == 0),
                                    stop=(ks == KS - 1),
                                )
                    for c in (2 * cg, 2 * cg + 1):
                        for pair in range(NP):
                            finish_qk(ps_q[(c, pair)], qT, pair, c, True, cp_i)
                            cp_i += 1

                # rnq partition-broadcast: K=2 indicator matmuls per chunk
                for pair in range(NP):
                    for c in range(NCH):
                        bc = ppQK.tile([P, CH], F32, tag="qk", name="bc")
                        nc.tensor.matmul(
                            bc[:], ind_sb[:],
                            rnq_sb[pair][:, c * CH:(c + 1) * CH],
                            start=True, stop=True,
                        )
                        copy_any(cp_i, rnqb[pair][:, c * CH:(c + 1) * CH], bc[:])
                        cp_i += 1

                # k chunks + norms + DMA block-transpose into kn
                for c in range(NCH):
                    ps_k = {}
                    for pair in range(NP):
                        ps_k[pair] = ppQK.tile(
                            [P, CH], F32, tag="qk", name=f"k{c}_{pair}"
                        )
                    for ks in range(KS):
                        for pair in range(NP):
                            nc.tensor.matmul(
                                ps_k[pair][:],
                                wk_sb[:, ks, pair * P:(pair + 1) * P],
                                x_sb[ks][:, c * CH:(c + 1) * CH],
                                start=(ks == 0),
                                stop=(ks == KS - 1),
                            )
                    for pair in range(NP):
                        finish_qk(ps_k[pair], kT, pair, c, False, cp_i)
                        cp_i += 1
                    nc.scalar.activation(
                        rnk_sb[:, 4 * c:4 * c + 4, :],
                        ssk_all[:, 4 * c:4 * c + 4, :],
                        AF.Abs_reciprocal_sqrt,
                    )
                    for pair in range(NP):
                        nc.sync.dma_start_transpose(
                            kn[pair][:, 4 * c:4 * c + 4, :],
                            kT[pair][:, c * CH:(c + 1) * CH],
                        )

                # v projection (+ fused beta logits) -> vtilde
                for tb in range(NT):
                    psv = ppV.tile([P, NV + HC], F32, tag="v", name=f"v{tb}")
                    for ks in range(KS):
                        nc.tensor.matmul(
                            psv[:],
                            x_sb[ks][:, tb * P:(tb + 1) * P],
                            wvb_sb[:, ks, :],
                            start=(ks == 0),
                            stop=(ks == KS - 1),
                        )
                    bl = ptmp.tile([P, HC], F32, tag="bl", name="bl")
                    nc.vector.tensor_tensor(
                        bl[:], psv[:, NV:], bbb_sb[:], OP.add
                    )
                    bs = ptmp.tile([P, HC], F32, tag="bs", name="bs")
                    nc.scalar.activation(bs[:], bl[:], AF.Sigmoid)
                    fac = ptmp.tile([P, HC], F32, tag="fac", name="fac")
                    nc.vector.tensor_tensor(
                        fac[:], bs[:], rnk_sb[:, tb, :], OP.mult
                    )
                    nc.vector.tensor_tensor(
                        vt[:, tb, :].rearrange("p (h e) -> p h e", e=DH),
                        psv[:, :NV].rearrange("p (h e) -> p h e", e=DH),
                        fac[:, :, None].to_broadcast((P, HC, DH)),
                        OP.mult,
                    )

            # ---------------- P2 + P3 ----------------
            with ExitStack() as p2:
                pst = p2.enter_context(tc.tile_pool(name="stbuf", bufs=6))
                pyout = p2.enter_context(tc.tile_pool(name="yout", bufs=2))
                psnap = p2.enter_context(tc.tile_pool(name="snap", bufs=1))
                ppST = p2.enter_context(
                    tc.tile_pool(name="ppST", bufs=2, space="PSUM")
                )
                ppO2 = p2.enter_context(
                    tc.tile_pool(name="ppO2", bufs=2, space="PSUM")
                )
                ppS = p2.enter_context(
                    tc.tile_pool(name="ppS", bufs=1, space="PSUM")
                )
                ppP3 = p2.enter_context(
                    tc.tile_pool(name="ppP3", bufs=2, space="PSUM")
                )

                s_ps = [ppS.tile([P, P], F32, tag=f"sps{p}", name=f"sps{p}")
                        for p in range(NP)]
                s_sb = [psnap.tile([P, P], BF16, tag=f"ssb{p}", name=f"ssb{p}")
                        for p in range(NP)]
                for pair in range(NP):
                    nc.vector.memset(s_ps[pair][:], 0.0)

                st_i = 0
                for c in range(NT):
                    lo, hi = c * P, (c + 1) * P
                    for pair in range(NP):
                        o2 = ppO2.tile([P, P], F32, tag="o2", name=f"o2_{c}_{pair}")
                        if c > 0:
                            # snapshot state (folds 0..c-1) and apply to q
                            nc.scalar.activation(
                                s_sb[pair][:], s_ps[pair][:], AF.Copy
                            )
                            nc.tensor.matmul(
                                o2[:], s_sb[pair][:], qT[pair][:, lo:hi],
                                start=True, stop=False,
                                skip_group_check=True,
                            )
                        for hh in range(2):
                            h = 2 * pair + hh
                            st_ps = ppST.tile([P, P], F32, tag="st", name="st")
                            nc.tensor.matmul(
                                st_ps[:],
                                kT[pair][64 * hh:64 * (hh + 1), lo:hi],
                                qT[pair][64 * hh:64 * (hh + 1), lo:hi],
                                start=True, stop=True,
                            )
                            st_sb = pst.tile([P, P], BF16, tag="st", name="st")
                            # causal mask folded into the mandatory copy
                            nc.vector.tensor_tensor(
                                st_sb[:], st_ps[:], mask_sb[:], OP.mult
                            )
                            st_i += 1
                            nc.tensor.matmul(
                                o2[64 * hh:64 * (hh + 1), :],
                                vt[:, c, h * DH:(h + 1) * DH],
                                st_sb[:],
                                start=(c == 0), stop=True,
                                tile_position=(0, 64 * hh),
                                skip_group_check=True,
                            )
                        nc.vector.tensor_tensor(
                            attnT[pair][:, lo:hi], o2[:],
                            rnqb[pair][:, lo:hi], OP.mult,
                        )
                        if c < NT - 1:
                            for hh in range(2):
                                h = 2 * pair + hh
                                nc.tensor.matmul(
                                    s_ps[pair][
                                        64 * hh:64 * (hh + 1),
                                        64 * hh:64 * (hh + 1),
                                    ],
                                    kn[pair][:, c, 64 * hh:64 * (hh + 1)],
                                    vt[:, c, h * DH:(h + 1) * DH],
                                    start=(c == 0), stop=(c == NT - 2),
                                    tile_position=(0, 64 * hh),
                                    skip_group_check=True,
                                )

                    # P3 for each completed 512-superchunk
                    if c % (CH // P) == (CH // P) - 1:
                        j = c // (CH // P)
                        yo = pyout.tile([P, D // P, CH], BF16, tag="yo", name="yo")
                        for m in range(D // P):
                            py = ppP3.tile([P, CH], F32, tag="py", name="py")
                            for pair in range(NP):
                                nc.tensor.matmul(
                                    py[:],
                                    wo_sb[:, pair, m * P:(m + 1) * P],
                                    attnT[pair][:, j * CH:(j + 1) * CH],
                                    start=(pair == 0),
                                    stop=(pair == NP - 1),
                                )
                            copy_any(m, yo[:, m, :], py[:])
                            if m % 2 == 1:
                                nc.sync.dma_start(
                                    yT.ap().rearrange("(m p) l -> p m l", p=P)[
                                        :, m - 1:m + 1, j * CH:(j + 1) * CH
                                    ],
                                    yo[:, m - 1:m + 1, :],
                                )

    nc.compile()
    return nc


def get_nc():
    if "nc" not in _CACHE:
        _CACHE["nc"] = _build_nc()
    return _CACHE["nc"]


def make_core_inputs(x, Wq, Wk, Wv, Wo, Wb, bb):
    """Build the 8 per-core input maps from full inputs."""
    BF = ml_dtypes.bfloat16
    x = np.asarray(x, dtype=np.float32)
    Wq = np.asarray(Wq, dtype=np.float32)
    Wk = np.asarray(Wk, dtype=np.float32)
    Wv = np.asarray(Wv, dtype=np.float32)
    Wo = np.asarray(Wo, dtype=np.float32)
    Wb = np.asarray(Wb, dtype=np.float32)
    bb = np.asarray(bb, dtype=np.float32)

    selm = np.zeros((P, 2), dtype=BF)
    selm[:64, 0] = 1.0
    selm[64:, 1] = 1.0
    indm = np.zeros((2, P), dtype=BF)
    indm[0, :64] = 1.0
    indm[1, 64:] = 1.0
    maskm = (np.arange(P)[:, None] <= np.arange(P)[None, :]).astype(np.float32)

    NV_G = HC * DH
    in_maps = []
    for core in range(NCORES):
        b, g = divmod(core, GROUPS)
        hs = slice(NV_G * g, NV_G * (g + 1))
        bs = slice(HC * g, HC * (g + 1))
        xTc = np.ascontiguousarray(x[b].T).astype(BF).reshape(KS, P, L)
        wqc = np.ascontiguousarray(Wq[:, hs]).astype(BF).reshape(KS, P, NP * P)
        wkc = np.ascontiguousarray(Wk[:, hs]).astype(BF).reshape(KS, P, NP * P)
        wvbc = np.ascontiguousarray(
            np.concatenate([Wv[:, hs], Wb[:, bs]], axis=1)
        ).astype(BF).reshape(KS, P, NV_G + HC)
        woc = np.ascontiguousarray(Wo[hs, :]).astype(BF).reshape(NP, P, D)
        bbbc = np.ascontiguousarray(np.tile(bb[bs][None, :], (P, 1)))
        in_maps.append(
            {
                "xT": xTc,
                "wq": wqc,
                "wk": wkc,
                "wvb": wvbc,
                "wo": woc,
                "sel": selm,
                "bbb": bbbc,
                "masks": maskm,
                "ind": indm,
            }
        )
    return in_maps


def kernel(x, Wq, Wk, Wv, Wo, Wb, bb):
    from concourse.bass_utils import run_bass_kernel_spmd

    nc = get_nc()
    in_maps = make_core_inputs(x, Wq, Wk, Wv, Wo, Wb, bb)
    try:
        res = run_bass_kernel_spmd(nc, in_maps, core_ids=list(range(NCORES)))
    except Exception:
        # transient NRT wedges clear on a fresh attempt; retry once
        res = run_bass_kernel_spmd(nc, in_maps, core_ids=list(range(NCORES)))
    B = 2
    y = np.zeros((B, L, D), dtype=np.float32)
    for core in range(NCORES):
        b = core // GROUPS
        y[b] += np.asarray(res.results[core]["yT"], dtype=np.float32).T
    return y


if __name__ == "__main__":
    rng = np.random.default_rng(0)
    ins = {
        "x": rng.standard_normal((2, L, D)).astype(np.float32),
        "Wq": (0.02 * rng.standard_normal((D, D))).astype(np.float32),
        "Wk": (0.02 * rng.standard_normal((D, D))).astype(np.float32),
        "Wv": (0.02 * rng.standard_normal((D, D))).astype(np.float32),
        "Wo": (0.02 * rng.standard_normal((D, D))).astype(np.float32),
        "Wb": (0.02 * rng.standard_normal((D, H))).astype(np.float32),
        "bb": np.zeros(H, dtype=np.float32),
    }
    out = kernel(**ins)
    print("kernel ran, out shape", out.shape, "mean abs", np.abs(out).mean())
